# revision 49
# baseline (speedup 1.0000x reference)
"""GINE message-passing kernel for Trainium2 (8 NeuronCores, SPMD).

Strategy (v3):
  - Host: sort edges by dst, shard by dst-range across 8 cores. Nodes are
    re-ordered by degree class S = ceil(deg/8) (slot-granular padding, ~13%
    pad instead of ~100% for fixed-DEG), each class padded to 128-node
    tiles, uniform across cores so one SPMD program serves all 8.
    Per-edge streams (xg = x[src]+b1 and edge attrs) are interleaved into a
    single DRAM stream in matmul-ready block layout (one DMA per 16K-edge
    super-block). Pad slots use xg = -1e9 so relu() zeroes them exactly.
  - Device phase 1 (per 4096-edge pair of blocks): two matmuls against a
    block-diagonal W1 (K=128 packs 16 chunks of 8 attr feats) -> edge
    embeddings in PSUM [128,512]; one DVE add (xg + emb -> bf16 msg); one
    ACT relu; one matmul against a block-sum matrix -> 8-edge partial sums
    [16,512] f32; per 2 pairs DMA the PSUM partials straight to DRAM (f32,
    no staging copy). PE instruction order is software-pipelined (emb runs
    2 pairs ahead of pp) to keep the tensor engine dense.
  - Device phase 2 (per 128-node tile of class S): one DMA loads the
    node's S partial slots AND its x row into one combo tile; single DVE
    strided reduce -> agg (ones column pre-set in a 4-deep ring), PE
    transpose, node MLP (bias via ones-row, K=17), write output. Host
    inverse-permutes rows at the end.
"""

import numpy as np
import ml_dtypes

import concourse.bacc as bacc
import concourse.bass as bass
import concourse.mybir as mybir
import concourse.tile as tile
from concourse.bass_utils import run_bass_kernel_spmd
from concourse.masks import make_identity

F = 16          # node feature dim
A = 8           # edge attr dim
O = 32          # output dim
GROUPS = 16     # chunks per block
CHUNK = 128     # edges per chunk (matmul K)
BLOCK_EDGES = GROUPS * CHUNK      # 2048 edges per emb matmul
SB_BLOCKS = 8                     # blocks per super-block
SB_EDGES = SB_BLOCKS * BLOCK_EDGES  # 16384
SLOT = 8        # edges per partial slot
GRP = 14        # phase-2 tiles per batched group

N_NODES = 100_000
N_CORES = 8

f32 = mybir.dt.float32
bf16 = mybir.dt.bfloat16
bf16_np = ml_dtypes.bfloat16

TRACE = False
TRACE_ALL = False
LAST_RESULTS = None


def _ceil_div(a, b):
    return -(-a // b)


def _host_prep(x, src, dst, edge_attr, lin1_b, n_cores):
    """Slot-granular degree-class packing; returns per-core arrays + meta."""
    n_nodes = x.shape[0]
    npc = n_nodes // n_cores
    order = np.argsort(dst, kind="stable")
    dsts = dst[order]
    srcs = src[order].astype(np.int64)
    counts = np.bincount(dst, minlength=n_nodes).astype(np.int64)
    edge_bounds = np.searchsorted(dsts, np.arange(0, n_nodes + 1, npc))

    # degree class per node: S = ceil(deg/SLOT), min 1
    S_node_all = np.maximum(1, _ceil_div(counts, SLOT))
    s_max = int(S_node_all.max())
    # per-class padded node counts, uniform across cores
    n_cls = np.zeros(s_max + 1, np.int64)
    for c in range(n_cores):
        lo = c * npc
        cnt = np.bincount(S_node_all[lo:lo + npc], minlength=s_max + 1)
        n_cls = np.maximum(n_cls, cnt)
    for s in range(1, s_max + 1):
        if n_cls[s]:
            n_cls[s] = _ceil_div(n_cls[s], 128) * 128

    NT_PAD = int(n_cls.sum())
    # per-tile class list and slot offsets (shared by all cores)
    tiles = []          # S per 128-node tile
    for s in range(1, s_max + 1):
        tiles += [s] * (int(n_cls[s]) // 128)
    so_tile = np.concatenate([[0], np.cumsum(np.array(tiles) * 128)])
    total_slots = int(so_tile[-1])
    E = _ceil_div(total_slots * SLOT, SB_EDGES) * SB_EDGES
    T = E // BLOCK_EDGES
    NSB = E // SB_EDGES
    NP8 = E // SLOT

    # phase-2 groups: consecutive same-S tiles, <= GRP per group
    groups = []         # (S, tile_start, ntiles)
    t0 = 0
    while t0 < len(tiles):
        s = tiles[t0]
        t1 = t0
        while t1 < len(tiles) and tiles[t1] == s and t1 - t0 < GRP:
            t1 += 1
        groups.append((s, t0, t1 - t0))
        t0 = t1

    xb = (x + lin1_b[None, :]).astype(np.float32)

    per_core = []
    for c in range(n_cores):
        lo = c * npc
        e0, e1 = int(edge_bounds[c]), int(edge_bounds[c + 1])
        deg = counts[lo:lo + npc]
        S_node = S_node_all[lo:lo + npc]
        # class-major stable node order + dummy padding
        perm = np.full(NT_PAD, -1, np.int64)
        so_node = np.zeros(npc, np.int64)   # slot offset per real node
        pos0 = 0
        slot0 = 0
        for s in range(1, s_max + 1):
            ids = np.nonzero(S_node == s)[0]
            perm[pos0:pos0 + len(ids)] = ids
            so_node[ids] = slot0 + np.arange(len(ids)) * s
            pos0 += int(n_cls[s])
            slot0 += int(n_cls[s]) * s

        # dense edge positions: rank within node + slot offset.
        # Slot id s maps to stream position via s = pair*512 + j*32 + h*16 + g
        # so each pair's PSUM flush is a 2D affine DMA (1KB runs).
        rank = np.arange(e1 - e0, dtype=np.int64) - np.repeat(
            np.cumsum(deg) - deg, deg)
        s = so_node[dsts[e0:e1] - lo] + rank // SLOT
        pair, rem = s // 512, s % 512
        j, h, g_ = rem // 32, (rem % 32) // 16, rem % 16
        pos = (pair * 4096 + h * 2048 + g_ * 128 + j * 8 + rank % SLOT)

        xgf = np.full((E, F), -1e9, np.float32)
        xgf[pos] = xb[srcs[e0:e1]]
        attr_pad = np.zeros((E, A), np.float32)
        attr_pad[pos] = edge_attr[order[e0:e1]]

        xg_resh = (xgf.reshape(T, GROUPS, CHUNK, F).transpose(0, 2, 1, 3)
                   .reshape(NSB, SB_BLOCKS, 128, GROUPS * F))
        attr_resh = (attr_pad.reshape(T, GROUPS, CHUNK, A)
                     .transpose(0, 1, 3, 2)
                     .reshape(NSB, SB_BLOCKS, 128, 128))
        stream = np.ascontiguousarray(
            xg_resh.transpose(0, 2, 1, 3)
            .reshape(NSB * 128, SB_BLOCKS * GROUPS * F).astype(bf16_np))
        stream_a = np.ascontiguousarray(
            attr_resh.transpose(0, 2, 1, 3)
            .reshape(NSB * 128, SB_BLOCKS * 128)
            .astype(ml_dtypes.float8_e4m3fn))

        xs = np.zeros((NT_PAD, F), bf16_np)
        real = perm >= 0
        xs[real] = x[lo + perm[real]]
        per_core.append(dict(stream=stream, stream_a=stream_a,
                             xs=xs, perm=perm))

    meta = dict(T=T, NSB=NSB, NT_PAD=NT_PAD, npc=npc, E=E, NP8=NP8,
                tiles=tiles, groups=groups, so_tile=so_tile)
    return per_core, meta


def _host_consts(lin1_w, nn_w, nn_b):
    w1blk = np.zeros((128, 2, 2, GROUPS, F), np.float32)
    for d in range(2):
        for g in range(GROUPS):
            w1blk[A * g:A * g + A, d, d, g, :] = lin1_w.T
    w1blk = w1blk.reshape(128, 2 * 2 * GROUPS * F)
    bsum = np.zeros((128, GROUPS), np.float32)
    bsum[np.arange(128), np.arange(128) // SLOT] = 1.0
    nnwx_1 = np.concatenate([nn_w.T, nn_b[None, :]], axis=0)
    nnwx = np.zeros((96, O), np.float32)
    for r in range(3):
        nnwx[32 * r:32 * r + F + 1] = nnwx_1
    return (w1blk.astype(ml_dtypes.float8_e4m3fn),
            bsum.astype(bf16_np),
            nnwx.astype(bf16_np))


def _build_nc(meta):
    NSB = meta["NSB"]
    NT_PAD = meta["NT_PAD"]
    NP8 = meta["NP8"]
    groups = meta["groups"]
    so_tile = meta["so_tile"]
    PAIRS = NSB * 4

    nc = bacc.Bacc("TRN2", target_bir_lowering=False, debug=False)
    st_d = nc.dram_tensor("stream", [NSB * 128, SB_BLOCKS * GROUPS * F],
                          bf16, kind="ExternalInput")
    sa_d = nc.dram_tensor("stream_a", [NSB * 128, SB_BLOCKS * 128],
                          mybir.dt.float8e4, kind="ExternalInput")
    xs_d = nc.dram_tensor("xs", [NT_PAD, F], bf16, kind="ExternalInput")
    w1_d = nc.dram_tensor("w1blk", [128, 4 * GROUPS * F],
                          mybir.dt.float8e4, kind="ExternalInput")
    bs_d = nc.dram_tensor("bsum", [128, GROUPS], bf16,
                          kind="ExternalInput")
    nw_d = nc.dram_tensor("nnwx", [96, O], bf16, kind="ExternalInput")
    out_d = nc.dram_tensor("out", [O, NT_PAD], f32, kind="ExternalOutput")

    st_v = st_d.rearrange("(s p) c -> s p c", p=128)
    sa_v = sa_d.rearrange("(s p) c -> s p c", p=128)
    relu = mybir.ActivationFunctionType.Relu

    with tile.TileContext(nc) as tc:
        with (
            tc.tile_pool(name="const", bufs=1) as cpool,
            tc.tile_pool(name="work", bufs=3) as wpool,
            tc.tile_pool(name="psum", bufs=2, space="PSUM") as ppool,
            tc.tile_pool(name="dram", bufs=1, space="DRAM") as dpool,
        ):
            P8_d = dpool.tile([NP8, F], bf16)
            w1 = cpool.tile([128, 4 * GROUPS * F], mybir.dt.float8e4)
            nc.sync.dma_start(w1[:], w1_d[:])
            bs = cpool.tile([128, GROUPS], bf16)
            nc.sync.dma_start(bs[:], bs_d[:])
            nw = cpool.tile([96, O], bf16)
            nc.sync.dma_start(nw[:], nw_d[:])
            ident = cpool.tile([128, 128], bf16)
            make_identity(nc, ident[:])
            aggbufs = []
            for i in range(2):
                ab = cpool.tile([128, GRP, 32], bf16, tag=f"agg{i}")
                nc.gpsimd.memset(ab[:, :, F:F + 1], 1.0)
                aggbufs.append(ab)

            # ---------------- phase 1 + interleaved phase 2 ----------
            pend = []          # [(msg, gpr)] awaiting their pp matmul
            msg2 = None
            pp3 = None
            pstage = None
            ppk = 0
            pp_g0 = 0
            flushno = 0
            gi_dma = 0         # next group to issue combo DMA for
            gi_cmp = 0         # next group to emit compute for
            gq = {}            # gi -> (combo view, ou tile)
            ti_global = 0

            def grp_rows_end(gi):
                S, t0, g = groups[gi]
                return int(so_tile[t0]) + g * 128 * S

            def issue_group_dma(gi):
                S, t0, g = groups[gi]
                W = (S + 1) * F
                combo = wpool.tile([128, GRP * W], bf16, tag="combo", bufs=6)
                cv = combo[:, :g * W].rearrange("p (t s f) -> p t s f",
                                                t=g, s=S + 1)
                row0 = int(so_tile[t0])
                nc.gpsimd.dma_start(
                    cv[:, :, 0:S, :],
                    P8_d[row0:row0 + g * 128 * S, :]
                    .rearrange("(t p s) f -> p t s f", p=128, s=S))
                nc.gpsimd.dma_start(
                    cv[:, :, S, :],
                    xs_d.rearrange("(n p) f -> n p f", p=128)[t0:t0 + g]
                    .rearrange("n p f -> p n f"))
                gq[gi] = cv

            def emit_group_compute(gi):
                nonlocal ti_global
                S, t0, g = groups[gi]
                cv = gq.pop(gi)
                ab = aggbufs[gi % 2]
                with nc.allow_low_precision(reason="bf16 agg"):
                    nc.vector.reduce_sum(
                        ab[:, 0:g, 0:F],
                        cv.rearrange("p t s f -> p t f s"),
                        axis=mybir.AxisListType.X)
                itG = wpool.tile([F + 1, GRP * 128], bf16, tag="it", bufs=2)
                for t in range(g):
                    trp_t = ppool.tile([F + 1, 128], bf16, tag="trp",
                                       bufs=1, name=f"trp{ti_global}")
                    trp = trp_t[:]
                    nc.tensor.transpose(trp, ab[:, t, 0:F + 1], ident[:])
                    if ti_global % 2 == 0:
                        nc.vector.tensor_copy(
                            itG[:, t * 128:(t + 1) * 128], trp)
                    else:
                        nc.scalar.activation(
                            itG[:, t * 128:(t + 1) * 128], trp,
                            mybir.ActivationFunctionType.Copy)
                    ti_global += 1
                ouT = wpool.tile([O, GRP * 128], f32, tag="ou", bufs=3)
                for c0 in range(0, g, 4):
                    c1 = min(c0 + 4, g)
                    opT = ppool.tile([O, 512], f32, tag="opT", bufs=2)
                    nc.tensor.matmul(
                        opT[:, 0:(c1 - c0) * 128],
                        nw[0:F + 1, :], itG[:, c0 * 128:c1 * 128],
                        start=True, stop=True)
                    nc.scalar.activation(
                        ouT[:, c0 * 128:c1 * 128],
                        opT[:, 0:(c1 - c0) * 128],
                        mybir.ActivationFunctionType.Copy)
                nc.sync.dma_start(
                    out_d[:, t0 * 128:(t0 + g) * 128],
                    ouT[:, :g * 128])


            def pump_phase2(covered_rows, force=False):
                nonlocal gi_dma, gi_cmp
                while gi_dma < len(groups) and (
                        force or grp_rows_end(gi_dma) <= covered_rows):
                    issue_group_dma(gi_dma)
                    gi_dma += 1
                while gi_cmp < gi_dma - (0 if force else 2):
                    emit_group_compute(gi_cmp)
                    gi_cmp += 1

            def emit_pp(m, gpr):
                nonlocal pp3, pstage, ppk, pp_g0, flushno
                if ppk == 0:
                    pp3 = ppool.tile([96, 512], f32, tag="pp3", bufs=2)
                    pstage = wpool.tile([96, 512], bf16, tag="pst", bufs=5)
                    pp_g0 = gpr
                nc.tensor.matmul(pp3[32 * ppk:32 * ppk + 16, :], bs[:], m[:],
                                 start=True, stop=True)
                ppk += 1
                if ppk == 3:
                    flush_pp()

            def flush_pp():
                nonlocal ppk, flushno
                if ppk == 0:
                    return
                if flushno % 2 == 0:
                    nc.vector.tensor_copy(pstage[:], pp3[:])
                else:
                    nc.scalar.activation(pstage[:], pp3[:],
                                         mybir.ActivationFunctionType.Copy)
                flushno += 1
                for k in range(ppk):
                    nc.sync.dma_start(
                        P8_d[(pp_g0 + k) * 512:(pp_g0 + k + 1) * 512, :]
                        .rearrange("(j x) f -> j (x f)", j=16),
                        pstage[32 * k:32 * k + 16, :])
                pump_phase2((pp_g0 + ppk) * 512)
                ppk = 0

            for sb in range(NSB):
                sbx = wpool.tile([128, SB_BLOCKS * GROUPS * F], bf16,
                                 tag="sbin", bufs=4)
                nc.gpsimd.dma_start(sbx[:], st_v[sb])
                sba = wpool.tile([128, SB_BLOCKS * 128], mybir.dt.float8e4,
                                 tag="sba", bufs=4)
                nc.gpsimd.dma_start(sba[:], sa_v[sb])
                for pr in range(4):
                    gpr = sb * 4 + pr
                    emb2 = ppool.tile([128, 2, 256], f32, tag="emb2", bufs=3)
                    nc.tensor.matmul(
                        emb2[:],
                        sba[:, pr * 256:(pr + 1) * 256]
                        .rearrange("p (d c) -> p d c", d=2),
                        w1[:].rearrange("p (d n) -> p d n", d=2),
                        start=True, stop=True,
                        perf_mode=mybir.MatmulPerfMode.DoubleRow)
                    if len(pend) >= 5:
                        emit_pp(*pend.pop(0))
                    k2 = gpr % 2
                    if k2 == 0:
                        msg2 = wpool.tile([128, 1024], bf16, tag="msg",
                                          bufs=4)
                    msgv = msg2[:, k2 * 512:(k2 + 1) * 512]
                    nc.vector.tensor_add(
                        msgv.rearrange("p (t c) -> p t c", t=2),
                        sbx[:, pr * 512:(pr + 1) * 512]
                        .rearrange("p (t c) -> p t c", t=2),
                        emb2[:])
                    if k2 == 1:
                        nc.scalar.activation(msg2[:], msg2[:], relu)
                    pend.append((msgv, gpr))
            while pend:
                emit_pp(*pend.pop(0))
            flush_pp()
            pump_phase2(10**18, force=True)

    nc.compile()
    return nc


def kernel(x, edge_index, edge_attr, lin1_w, lin1_b, nn_w, nn_b):
    x = np.asarray(x, np.float32)
    edge_index = np.asarray(edge_index)
    edge_attr = np.asarray(edge_attr, np.float32)
    lin1_w = np.asarray(lin1_w, np.float32)
    lin1_b = np.asarray(lin1_b, np.float32)
    nn_w = np.asarray(nn_w, np.float32)
    nn_b = np.asarray(nn_b, np.float32)

    src = np.asarray(edge_index[0], np.int64)
    dst = np.asarray(edge_index[1], np.int64)
    per_core, meta = _host_prep(x, src, dst, edge_attr, lin1_b, N_CORES)
    w1blk, bsum, nnwx = _host_consts(lin1_w, nn_w, nn_b)

    nc = _build_nc(meta)

    in_maps = []
    for c in range(N_CORES):
        pc = per_core[c]
        in_maps.append({
            "stream": pc["stream"], "stream_a": pc["stream_a"],
            "xs": pc["xs"],
            "w1blk": w1blk, "bsum": bsum, "nnwx": nnwx,
        })
    global LAST_RESULTS
    res = run_bass_kernel_spmd(
        nc, in_maps, core_ids=list(range(N_CORES)), trace=TRACE,
        **({"stitch_traces": True, "trace_cores": list(range(N_CORES))}
           if TRACE_ALL else {}))
    LAST_RESULTS = res
    npc = meta["npc"]
    out = np.empty((N_NODES, O), np.float32)
    for c in range(N_CORES):
        perm = per_core[c]["perm"]
        real = perm >= 0
        out[c * npc + perm[real]] = res.results[c]["out"][:, real].T
    return np.ascontiguousarray(out, dtype=np.float32)


# revision 50
# speedup vs baseline: 1.0224x; 1.0224x over previous
"""GINE message-passing kernel for Trainium2 (8 NeuronCores, SPMD).

Strategy (v3):
  - Host: sort edges by dst, shard by dst-range across 8 cores. Nodes are
    re-ordered by degree class S = ceil(deg/8) (slot-granular padding, ~13%
    pad instead of ~100% for fixed-DEG), each class padded to 128-node
    tiles, uniform across cores so one SPMD program serves all 8.
    Per-edge streams (xg = x[src]+b1 and edge attrs) are interleaved into a
    single DRAM stream in matmul-ready block layout (one DMA per 16K-edge
    super-block). Pad slots use xg = -1e9 so relu() zeroes them exactly.
  - Device phase 1 (per 4096-edge pair of blocks): two matmuls against a
    block-diagonal W1 (K=128 packs 16 chunks of 8 attr feats) -> edge
    embeddings in PSUM [128,512]; one DVE add (xg + emb -> bf16 msg); one
    ACT relu; one matmul against a block-sum matrix -> 8-edge partial sums
    [16,512] f32; per 2 pairs DMA the PSUM partials straight to DRAM (f32,
    no staging copy). PE instruction order is software-pipelined (emb runs
    2 pairs ahead of pp) to keep the tensor engine dense.
  - Device phase 2 (per 128-node tile of class S): one DMA loads the
    node's S partial slots AND its x row into one combo tile; single DVE
    strided reduce -> agg (ones column pre-set in a 4-deep ring), PE
    transpose, node MLP (bias via ones-row, K=17), write output. Host
    inverse-permutes rows at the end.
"""

import numpy as np
import ml_dtypes

import concourse.bacc as bacc
import concourse.bass as bass
import concourse.mybir as mybir
import concourse.tile as tile
from concourse.bass_utils import run_bass_kernel_spmd
from concourse.masks import make_identity

F = 16          # node feature dim
A = 8           # edge attr dim
O = 32          # output dim
GROUPS = 16     # chunks per block
CHUNK = 128     # edges per chunk (matmul K)
BLOCK_EDGES = GROUPS * CHUNK      # 2048 edges per emb matmul
SB_BLOCKS = 8                     # blocks per super-block
SB_EDGES = SB_BLOCKS * BLOCK_EDGES  # 16384
SLOT = 8        # edges per partial slot
GRP = 10        # phase-2 tiles per batched group

N_NODES = 100_000
N_CORES = 8

f32 = mybir.dt.float32
bf16 = mybir.dt.bfloat16
bf16_np = ml_dtypes.bfloat16

TRACE = False
TRACE_ALL = False
LAST_RESULTS = None


def _ceil_div(a, b):
    return -(-a // b)


def _host_prep(x, src, dst, edge_attr, lin1_b, n_cores):
    """Slot-granular degree-class packing; returns per-core arrays + meta."""
    n_nodes = x.shape[0]
    npc = n_nodes // n_cores
    order = np.argsort(dst, kind="stable")
    dsts = dst[order]
    srcs = src[order].astype(np.int64)
    counts = np.bincount(dst, minlength=n_nodes).astype(np.int64)
    edge_bounds = np.searchsorted(dsts, np.arange(0, n_nodes + 1, npc))

    # degree class per node: S = ceil(deg/SLOT), min 1
    S_node_all = np.maximum(1, _ceil_div(counts, SLOT))
    s_max = int(S_node_all.max())
    # per-class padded node counts, uniform across cores
    n_cls = np.zeros(s_max + 1, np.int64)
    for c in range(n_cores):
        lo = c * npc
        cnt = np.bincount(S_node_all[lo:lo + npc], minlength=s_max + 1)
        n_cls = np.maximum(n_cls, cnt)
    for s in range(1, s_max + 1):
        if n_cls[s]:
            n_cls[s] = _ceil_div(n_cls[s], 128) * 128

    NT_PAD = int(n_cls.sum())
    # per-tile class list and slot offsets (shared by all cores)
    tiles = []          # S per 128-node tile
    for s in range(1, s_max + 1):
        tiles += [s] * (int(n_cls[s]) // 128)
    so_tile = np.concatenate([[0], np.cumsum(np.array(tiles) * 128)])
    total_slots = int(so_tile[-1])
    E = _ceil_div(total_slots * SLOT, SB_EDGES) * SB_EDGES
    T = E // BLOCK_EDGES
    NSB = E // SB_EDGES
    NP8 = E // SLOT

    # phase-2 groups: consecutive same-S tiles, <= GRP per group
    groups = []         # (S, tile_start, ntiles)
    t0 = 0
    while t0 < len(tiles):
        s = tiles[t0]
        t1 = t0
        while t1 < len(tiles) and tiles[t1] == s and t1 - t0 < GRP:
            t1 += 1
        groups.append((s, t0, t1 - t0))
        t0 = t1

    xb = (x + lin1_b[None, :]).astype(np.float32)

    per_core = []
    for c in range(n_cores):
        lo = c * npc
        e0, e1 = int(edge_bounds[c]), int(edge_bounds[c + 1])
        deg = counts[lo:lo + npc]
        S_node = S_node_all[lo:lo + npc]
        # class-major stable node order + dummy padding
        perm = np.full(NT_PAD, -1, np.int64)
        so_node = np.zeros(npc, np.int64)   # slot offset per real node
        pos0 = 0
        slot0 = 0
        for s in range(1, s_max + 1):
            ids = np.nonzero(S_node == s)[0]
            perm[pos0:pos0 + len(ids)] = ids
            so_node[ids] = slot0 + np.arange(len(ids)) * s
            pos0 += int(n_cls[s])
            slot0 += int(n_cls[s]) * s

        # dense edge positions: rank within node + slot offset.
        # Slot id s maps to stream position via s = pair*512 + j*32 + h*16 + g
        # so each pair's PSUM flush is a 2D affine DMA (1KB runs).
        rank = np.arange(e1 - e0, dtype=np.int64) - np.repeat(
            np.cumsum(deg) - deg, deg)
        s = so_node[dsts[e0:e1] - lo] + rank // SLOT
        pair, rem = s // 512, s % 512
        j, h, g_ = rem // 32, (rem % 32) // 16, rem % 16
        pos = (pair * 4096 + h * 2048 + g_ * 128 + j * 8 + rank % SLOT)

        xgf = np.full((E, F), -1e9, np.float32)
        xgf[pos] = xb[srcs[e0:e1]]
        attr_pad = np.zeros((E, A), np.float32)
        attr_pad[pos] = edge_attr[order[e0:e1]]

        xg_resh = (xgf.reshape(T, GROUPS, CHUNK, F).transpose(0, 2, 1, 3)
                   .reshape(NSB, SB_BLOCKS, 128, GROUPS * F))
        attr_resh = (attr_pad.reshape(T, GROUPS, CHUNK, A)
                     .transpose(0, 1, 3, 2)
                     .reshape(NSB, SB_BLOCKS, 128, 128))
        stream = np.ascontiguousarray(
            xg_resh.transpose(0, 2, 1, 3)
            .reshape(NSB * 128, SB_BLOCKS * GROUPS * F).astype(bf16_np))
        stream_a = np.ascontiguousarray(
            attr_resh.transpose(0, 2, 1, 3)
            .reshape(NSB * 128, SB_BLOCKS * 128)
            .astype(ml_dtypes.float8_e4m3fn))

        xs = np.zeros((NT_PAD, F), bf16_np)
        real = perm >= 0
        xs[real] = x[lo + perm[real]]
        per_core.append(dict(stream=stream, stream_a=stream_a,
                             xs=xs, perm=perm))

    meta = dict(T=T, NSB=NSB, NT_PAD=NT_PAD, npc=npc, E=E, NP8=NP8,
                tiles=tiles, groups=groups, so_tile=so_tile)
    return per_core, meta


def _host_consts(lin1_w, nn_w, nn_b):
    w1blk = np.zeros((128, 2, 2, GROUPS, F), np.float32)
    for d in range(2):
        for g in range(GROUPS):
            w1blk[A * g:A * g + A, d, d, g, :] = lin1_w.T
    w1blk = w1blk.reshape(128, 2 * 2 * GROUPS * F)
    bsum = np.zeros((128, GROUPS), np.float32)
    bsum[np.arange(128), np.arange(128) // SLOT] = 1.0
    nnwx_1 = np.concatenate([nn_w.T, nn_b[None, :]], axis=0)
    nnwx = np.zeros((96, O), np.float32)
    for r in range(3):
        nnwx[32 * r:32 * r + F + 1] = nnwx_1
    return (w1blk.astype(ml_dtypes.float8_e4m3fn),
            bsum.astype(bf16_np),
            nnwx.astype(bf16_np))


def _build_nc(meta):
    NSB = meta["NSB"]
    NT_PAD = meta["NT_PAD"]
    NP8 = meta["NP8"]
    groups = meta["groups"]
    so_tile = meta["so_tile"]
    PAIRS = NSB * 4

    nc = bacc.Bacc("TRN2", target_bir_lowering=False, debug=False)
    st_d = nc.dram_tensor("stream", [NSB * 128, SB_BLOCKS * GROUPS * F],
                          bf16, kind="ExternalInput")
    sa_d = nc.dram_tensor("stream_a", [NSB * 128, SB_BLOCKS * 128],
                          mybir.dt.float8e4, kind="ExternalInput")
    xs_d = nc.dram_tensor("xs", [NT_PAD, F], bf16, kind="ExternalInput")
    w1_d = nc.dram_tensor("w1blk", [128, 4 * GROUPS * F],
                          mybir.dt.float8e4, kind="ExternalInput")
    bs_d = nc.dram_tensor("bsum", [128, GROUPS], bf16,
                          kind="ExternalInput")
    nw_d = nc.dram_tensor("nnwx", [96, O], bf16, kind="ExternalInput")
    out_d = nc.dram_tensor("out", [O, NT_PAD], f32, kind="ExternalOutput")

    st_v = st_d.rearrange("(s p) c -> s p c", p=128)
    sa_v = sa_d.rearrange("(s p) c -> s p c", p=128)
    relu = mybir.ActivationFunctionType.Relu

    with tile.TileContext(nc) as tc:
        with (
            tc.tile_pool(name="const", bufs=1) as cpool,
            tc.tile_pool(name="work", bufs=3) as wpool,
            tc.tile_pool(name="psum", bufs=2, space="PSUM") as ppool,
            tc.tile_pool(name="dram", bufs=1, space="DRAM") as dpool,
        ):
            P8_d = dpool.tile([NP8, F], bf16)
            w1 = cpool.tile([128, 4 * GROUPS * F], mybir.dt.float8e4)
            nc.sync.dma_start(w1[:], w1_d[:])
            bs = cpool.tile([128, GROUPS], bf16)
            nc.sync.dma_start(bs[:], bs_d[:])
            nw = cpool.tile([96, O], bf16)
            nc.sync.dma_start(nw[:], nw_d[:])
            ident = cpool.tile([128, 128], bf16)
            make_identity(nc, ident[:])
            aggbufs = []
            for i in range(2):
                ab = cpool.tile([128, GRP, 32], bf16, tag=f"agg{i}")
                nc.gpsimd.memset(ab[:, :, F:F + 1], 1.0)
                aggbufs.append(ab)

            # ---------------- phase 1 + interleaved phase 2 ----------
            pend = []          # [(msg, gpr)] awaiting their pp matmul
            msg2 = None
            pp3 = None
            pstage = None
            ppk = 0
            pp_g0 = 0
            flushno = 0
            gi_dma = 0         # next group to issue combo DMA for
            gi_cmp = 0         # next group to emit compute for
            gq = {}            # gi -> (combo view, ou tile)
            ti_global = 0

            def grp_rows_end(gi):
                S, t0, g = groups[gi]
                return int(so_tile[t0]) + g * 128 * S

            def issue_group_dma(gi):
                S, t0, g = groups[gi]
                W = (S + 1) * F
                combo = wpool.tile([128, GRP * W], bf16, tag="combo", bufs=6)
                cv = combo[:, :g * W].rearrange("p (t s f) -> p t s f",
                                                t=g, s=S + 1)
                row0 = int(so_tile[t0])
                nc.gpsimd.dma_start(
                    cv[:, :, 0:S, :],
                    P8_d[row0:row0 + g * 128 * S, :]
                    .rearrange("(t p s) f -> p t s f", p=128, s=S))
                nc.gpsimd.dma_start(
                    cv[:, :, S, :],
                    xs_d.rearrange("(n p) f -> n p f", p=128)[t0:t0 + g]
                    .rearrange("n p f -> p n f"))
                gq[gi] = cv

            def emit_group_compute(gi):
                nonlocal ti_global
                S, t0, g = groups[gi]
                cv = gq.pop(gi)
                ab = aggbufs[gi % 2]
                with nc.allow_low_precision(reason="bf16 agg"):
                    nc.vector.reduce_sum(
                        ab[:, 0:g, 0:F],
                        cv.rearrange("p t s f -> p t f s"),
                        axis=mybir.AxisListType.X)
                itG = wpool.tile([F + 1, GRP * 128], bf16, tag="it", bufs=2)
                for t in range(g):
                    trp_t = ppool.tile([F + 1, 128], bf16, tag="trp",
                                       bufs=1, name=f"trp{ti_global}")
                    trp = trp_t[:]
                    nc.tensor.transpose(trp, ab[:, t, 0:F + 1], ident[:])
                    if ti_global % 2 == 0:
                        nc.vector.tensor_copy(
                            itG[:, t * 128:(t + 1) * 128], trp)
                    else:
                        nc.scalar.activation(
                            itG[:, t * 128:(t + 1) * 128], trp,
                            mybir.ActivationFunctionType.Copy)
                    ti_global += 1
                ouT = wpool.tile([O, GRP * 128], f32, tag="ou", bufs=3)
                for c0 in range(0, g, 4):
                    c1 = min(c0 + 4, g)
                    opT = ppool.tile([O, 512], f32, tag="opT", bufs=2)
                    nc.tensor.matmul(
                        opT[:, 0:(c1 - c0) * 128],
                        nw[0:F + 1, :], itG[:, c0 * 128:c1 * 128],
                        start=True, stop=True)
                    nc.scalar.activation(
                        ouT[:, c0 * 128:c1 * 128],
                        opT[:, 0:(c1 - c0) * 128],
                        mybir.ActivationFunctionType.Copy)
                nc.sync.dma_start(
                    out_d[:, t0 * 128:(t0 + g) * 128],
                    ouT[:, :g * 128])


            def pump_phase2(covered_rows, force=False):
                nonlocal gi_dma, gi_cmp
                while gi_dma < len(groups) and (
                        force or grp_rows_end(gi_dma) <= covered_rows):
                    issue_group_dma(gi_dma)
                    gi_dma += 1
                while gi_cmp < gi_dma - (0 if force else 2):
                    emit_group_compute(gi_cmp)
                    gi_cmp += 1

            def emit_pp(m, gpr):
                nonlocal pp3, pstage, ppk, pp_g0, flushno
                if ppk == 0:
                    pp3 = ppool.tile([96, 512], f32, tag="pp3", bufs=2)
                    pstage = wpool.tile([96, 512], bf16, tag="pst", bufs=5)
                    pp_g0 = gpr
                nc.tensor.matmul(pp3[32 * ppk:32 * ppk + 16, :], bs[:], m[:],
                                 start=True, stop=True)
                ppk += 1
                if ppk == 3:
                    flush_pp()

            def flush_pp():
                nonlocal ppk, flushno
                if ppk == 0:
                    return
                if flushno % 2 == 0:
                    nc.vector.tensor_copy(pstage[:], pp3[:])
                else:
                    nc.scalar.activation(pstage[:], pp3[:],
                                         mybir.ActivationFunctionType.Copy)
                flushno += 1
                for k in range(ppk):
                    nc.sync.dma_start(
                        P8_d[(pp_g0 + k) * 512:(pp_g0 + k + 1) * 512, :]
                        .rearrange("(j x) f -> j (x f)", j=16),
                        pstage[32 * k:32 * k + 16, :])
                pump_phase2((pp_g0 + ppk) * 512)
                ppk = 0

            for sb in range(NSB):
                sbx = wpool.tile([128, SB_BLOCKS * GROUPS * F], bf16,
                                 tag="sbin", bufs=4)
                nc.gpsimd.dma_start(sbx[:], st_v[sb])
                sba = wpool.tile([128, SB_BLOCKS * 128], mybir.dt.float8e4,
                                 tag="sba", bufs=4)
                nc.gpsimd.dma_start(sba[:], sa_v[sb])
                for pr in range(4):
                    gpr = sb * 4 + pr
                    emb2 = ppool.tile([128, 2, 256], f32, tag="emb2", bufs=3)
                    nc.tensor.matmul(
                        emb2[:],
                        sba[:, pr * 256:(pr + 1) * 256]
                        .rearrange("p (d c) -> p d c", d=2),
                        w1[:].rearrange("p (d n) -> p d n", d=2),
                        start=True, stop=True,
                        perf_mode=mybir.MatmulPerfMode.DoubleRow)
                    if len(pend) >= 5:
                        emit_pp(*pend.pop(0))
                    k2 = gpr % 2
                    if k2 == 0:
                        msg2 = wpool.tile([128, 1024], bf16, tag="msg",
                                          bufs=4)
                    msgv = msg2[:, k2 * 512:(k2 + 1) * 512]
                    nc.vector.tensor_add(
                        msgv.rearrange("p (t c) -> p t c", t=2),
                        sbx[:, pr * 512:(pr + 1) * 512]
                        .rearrange("p (t c) -> p t c", t=2),
                        emb2[:])
                    if k2 == 1:
                        nc.scalar.activation(msg2[:], msg2[:], relu)
                    pend.append((msgv, gpr))
            while pend:
                emit_pp(*pend.pop(0))
            flush_pp()
            pump_phase2(10**18, force=True)

    nc.compile()
    return nc


def kernel(x, edge_index, edge_attr, lin1_w, lin1_b, nn_w, nn_b):
    x = np.asarray(x, np.float32)
    edge_index = np.asarray(edge_index)
    edge_attr = np.asarray(edge_attr, np.float32)
    lin1_w = np.asarray(lin1_w, np.float32)
    lin1_b = np.asarray(lin1_b, np.float32)
    nn_w = np.asarray(nn_w, np.float32)
    nn_b = np.asarray(nn_b, np.float32)

    src = np.asarray(edge_index[0], np.int64)
    dst = np.asarray(edge_index[1], np.int64)
    per_core, meta = _host_prep(x, src, dst, edge_attr, lin1_b, N_CORES)
    w1blk, bsum, nnwx = _host_consts(lin1_w, nn_w, nn_b)

    nc = _build_nc(meta)

    in_maps = []
    for c in range(N_CORES):
        pc = per_core[c]
        in_maps.append({
            "stream": pc["stream"], "stream_a": pc["stream_a"],
            "xs": pc["xs"],
            "w1blk": w1blk, "bsum": bsum, "nnwx": nnwx,
        })
    global LAST_RESULTS
    res = run_bass_kernel_spmd(
        nc, in_maps, core_ids=list(range(N_CORES)), trace=TRACE,
        **({"stitch_traces": True, "trace_cores": list(range(N_CORES))}
           if TRACE_ALL else {}))
    LAST_RESULTS = res
    npc = meta["npc"]
    out = np.empty((N_NODES, O), np.float32)
    for c in range(N_CORES):
        perm = per_core[c]["perm"]
        real = perm >= 0
        out[c * npc + perm[real]] = res.results[c]["out"][:, real].T
    return np.ascontiguousarray(out, dtype=np.float32)
